# revision 6
# baseline (speedup 1.0000x reference)
"""Trainium2 Bass kernel for nn_DinoGazeSpade (segment_reduce + repaint).

reference semantics:
  seg_feat = mask[:, ::14, ::14]                       # nearest-downsample to 28x28
  seg_avg[b, s, :] = mean of feat pixels with seg==s   # scatter_mean over B*128 segments
  out[b, :, hi, wi] = seg_avg[b, mask[b, hi, wi], :]   # repaint at full res

Sharding: 8 cores = 2 batches x 4 row-slices of the 392-row full-res output.
Each core computes its batch's seg table (tiny) and paints its 98-row
slice. The paint is a one-hot(segment) x seg_sums matmul on the tensor
engine, which directly produces the channel-major output layout. The 1/count
mean scaling is folded into the one-hot values, so the seg table is just the
raw bf16 segment sums. All matmuls run single-plane bf16: the 2e-2 rel-err
budget dwarfs bf16 rounding (~4e-3 end to end).
"""

import numpy as np
import ml_dtypes
from contextlib import ExitStack

import concourse.bass as bass
import concourse.tile as tile
from concourse import bacc, mybir
from concourse.bass_utils import run_bass_kernel_spmd

# problem shape (hardcoded per contract)
B, C, Hp, Wp = 2, 768, 28, 28
Hi, Wi = 392, 392
S = 128                    # segments per image
N_CORES = 8
ROWS = Hi // 4             # 98 full-res rows per core
NPIX = ROWS * Wi           # 38416 pixels per core
NPATCH = Hp * Wp           # 784 patch pixels
PCHUNK = 112               # 784 = 7 * 112 patch-pixel chunks (partition dim)
NCH = NPATCH // PCHUNK     # 7 chunks
PTILE = 512                # paint pixel tile (one PSUM bank)
GROUP = 3 * PTILE          # 1536 pixels per paint group
NGROUP = NPIX // GROUP     # 25 full groups
REM = NPIX - NGROUP * GROUP  # 16 remainder pixels
CT = C // 128              # 6 channel tiles

f32 = mybir.dt.float32
bf16 = mybir.dt.bfloat16
i32 = mybir.dt.int32

_CACHED_NC = None


def _build_nc():
    nc = bacc.Bacc()
    fpk_hbm = nc.dram_tensor("fpk", [PCHUNK, NCH, C], bf16, kind="ExternalInput")
    pmk_hbm = nc.dram_tensor("pmk", [PCHUNK, NCH], f32, kind="ExternalInput")
    mask_hbm = nc.dram_tensor("mask", [1, NPIX], bf16, kind="ExternalInput")
    out_hbm = nc.dram_tensor("out", [C, NPIX], bf16, kind="ExternalOutput")

    with tile.TileContext(nc) as tc, ExitStack() as ctx:
        const = ctx.enter_context(tc.tile_pool(name="const", bufs=1))
        segp = ctx.enter_context(tc.tile_pool(name="segp", bufs=1))
        # paint-phase SBUF + one-hot PSUM pools created BEFORE the scatter
        # scratch pool so one-hot building can overlap the scatter phase
        sbB = ctx.enter_context(tc.tile_pool(name="sbB", bufs=6))
        osb = ctx.enter_context(tc.tile_pool(name="osb", bufs=10))
        psB = ctx.enter_context(tc.tile_pool(name="psB", bufs=2, space="PSUM"))

        # ---- input loads: pmk + feature chunks first (scatter critical
        # path), then the full-res mask (needed a bit later by the paint) ----
        ld = ctx.enter_context(tc.tile_pool(name="ld", bufs=1))
        pmk = ld.tile([PCHUNK, NCH], f32)
        nc.gpsimd.dma_start(out=pmk[:], in_=pmk_hbm[:, :])
        fsb = ld.tile([PCHUNK, NCH, C], bf16)
        # chunk 0 lands first so the first scatter matmul can start early
        nc.gpsimd.dma_start(out=fsb[:, 0:2, :], in_=fpk_hbm[:, 0:2, :])
        nc.gpsimd.dma_start(out=fsb[:, 2:NCH, :], in_=fpk_hbm[:, 2:NCH, :])
        mask_sb = ld.tile([1, NPIX], bf16)
        nc.gpsimd.dma_start(out=mask_sb[:], in_=mask_hbm[:, :])

        # ---- constants ----
        iota_pi = const.tile([128, 1], i32)           # partition index
        nc.gpsimd.iota(iota_pi[:], [[0, 1]], channel_multiplier=1)
        iota_pf = const.tile([128, 1], f32)
        nc.vector.tensor_copy(iota_pf[:], iota_pi[:])
        iota_ri = const.tile([128, 128], i32)         # free-dim index (same per partition)
        nc.gpsimd.iota(iota_ri[:], [[1, 128]], channel_multiplier=0)
        iota_rf = const.tile([128, 128], f32)
        nc.vector.tensor_copy(iota_rf[:], iota_ri[:])
        ones_bf = const.tile([1, 128], bf16)
        nc.vector.memset(ones_bf[:], 1.0)
        ones_col = const.tile([128, 1], bf16)
        nc.vector.memset(ones_col[:], 1.0)

        # ---- phase A: scatter-sum over patch pixels -> seg sums [S=128, C]
        # (the 1/count scaling is folded into the paint one-hots) ----
        seg_bf = segp.tile([128, C], bf16)
        rcp = segp.tile([128, 1], f32)

        psA_cm = tc.tile_pool(name="psA", bufs=1, space="PSUM")
        with tc.tile_pool(name="sbA", bufs=2) as sbA, psA_cm as psA:
            sums0 = psA.tile([128, 384], f32, tag="sums0", name="sums0")
            sums1 = psA.tile([128, 384], f32, tag="sums1", name="sums1")
            cnt_ps = psA.tile([128, 1], f32, tag="cnt", name="cnt")
            for k in range(NCH):
                oh = sbA.tile([PCHUNK, 128], bf16, tag="ohp")
                nc.vector.tensor_tensor(
                    out=oh[:], in0=iota_rf[0:PCHUNK, :],
                    in1=pmk[:, k:k + 1].to_broadcast([PCHUNK, 128]),
                    op=mybir.AluOpType.is_equal,
                )
                first, last = k == 0, k == NCH - 1
                nc.tensor.matmul(sums0[:], lhsT=oh[:], rhs=fsb[:, k, 0:384],
                                 start=first, stop=last)
                nc.tensor.matmul(sums1[:], lhsT=oh[:], rhs=fsb[:, k, 384:768],
                                 start=first, stop=last)
                nc.tensor.matmul(cnt_ps[:], lhsT=oh[:], rhs=ones_col[0:PCHUNK, :],
                                 start=first, stop=last)

            # rcp = 1 / max(cnt, 1); empty segments have sums == 0 so avg == 0
            cnt_sb = sbA.tile([128, 1], f32)
            nc.vector.tensor_scalar_max(cnt_sb[:], cnt_ps[:], 1.0)
            nc.vector.reciprocal(rcp[:], cnt_sb[:])
            # raw sums -> bf16 paint table
            nc.vector.tensor_copy(seg_bf[:, 0:384], sums0[:])
            nc.vector.tensor_copy(seg_bf[:, 384:768], sums1[:])

        # ---- phase B: paint full-res pixels ----
        # psO tiles span 3 PSUM banks: the 3 per-tile matmuls land in one
        # psO tile and drain with a single wide cast (amortizes the ~300ns
        # fixed cost per DVE/ACT instruction). psB 2x1 + psO 2x3 = 8 banks.
        psO = ctx.enter_context(tc.tile_pool(name="psO", bufs=2, space="PSUM"))
        cast_i = [0]

        def paint(pix0, sizes):
            # one group: pixels [pix0, pix0+sum(sizes)), one tile per size
            npx = sum(sizes)
            offs = [sum(sizes[:t]) for t in range(len(sizes))]
            ohs = []
            for t, sz in enumerate(sizes):
                bc = psB.tile([128, sz], f32, tag="bc", name="bc")
                nc.tensor.matmul(
                    bc[:], lhsT=ones_bf[:],
                    rhs=mask_sb[0:1, pix0 + offs[t]:pix0 + offs[t] + sz],
                    start=True, stop=True,
                )
                # one-hot scaled by 1/count: out = (bc == iota_p) * rcp
                oh = sbB.tile([128, sz], bf16, tag="ohb", name="ohb")
                nc.vector.tensor_scalar(
                    out=oh[:], in0=bc[:], scalar1=iota_pf[:], scalar2=rcp[:],
                    op0=mybir.AluOpType.is_equal, op1=mybir.AluOpType.mult,
                )
                ohs.append(oh)
            for c in range(CT):
                ob = osb.tile([128, npx], bf16, tag="ob", name="ob")
                op = psO.tile([128, npx], f32, tag="op", name="op")
                for t in range(len(sizes)):
                    nc.tensor.matmul(op[:, offs[t]:offs[t] + sizes[t]],
                                     lhsT=seg_bf[:, c * 128:(c + 1) * 128],
                                     rhs=ohs[t][:], start=True, stop=True)
                # one wide psum->sbuf cast; DVE also builds one-hots, so it
                # gets ~1/3 of the drain and ACT the rest
                if cast_i[0] % 9 < 3:
                    nc.vector.tensor_copy(ob[:], op[:])
                else:
                    nc.scalar.copy(ob[:], op[:])
                cast_i[0] += 1
                nc.sync.dma_start(
                    out=out_hbm[c * 128:(c + 1) * 128, pix0:pix0 + npx], in_=ob[:]
                )

        for g in range(NGROUP):
            paint(g * GROUP, [PTILE] * 3)
        if REM:
            paint(NGROUP * GROUP, [REM])

    nc.compile()
    return nc


def make_in_maps(F_semantic_patches, segmentation_mask):
    F = np.asarray(F_semantic_patches, dtype=np.float32)
    M = np.asarray(segmentation_mask)
    in_maps = []
    for core in range(N_CORES):
        b, q = divmod(core, 4)
        feat = F[b].reshape(C, NPATCH).T                               # [784, 768]
        # [p, k, c] so one DMA lands chunk k on partitions
        fpk = np.ascontiguousarray(
            feat.reshape(NCH, PCHUNK, C).transpose(1, 0, 2)
        ).astype(ml_dtypes.bfloat16)
        pmk = np.ascontiguousarray(
            M[b, ::Hi // Hp, ::Wi // Wp].reshape(NCH, PCHUNK).T
        ).astype(np.float32)
        mask = np.ascontiguousarray(
            M[b, q * ROWS:(q + 1) * ROWS, :].reshape(1, NPIX)
        ).astype(ml_dtypes.bfloat16)
        in_maps.append({"fpk": fpk, "pmk": pmk, "mask": mask})
    return in_maps


def kernel(F_semantic_patches: np.ndarray, segmentation_mask: np.ndarray) -> np.ndarray:
    global _CACHED_NC
    if _CACHED_NC is None:
        _CACHED_NC = _build_nc()
    nc = _CACHED_NC

    in_maps = make_in_maps(F_semantic_patches, segmentation_mask)

    res = run_bass_kernel_spmd(nc, in_maps, core_ids=list(range(N_CORES)))

    out = np.empty((B, C, Hi, Wi), dtype=np.float32)
    for core in range(N_CORES):
        b, q = divmod(core, 4)
        out[b, :, q * ROWS:(q + 1) * ROWS, :] = (
            res.results[core]["out"].astype(np.float32).reshape(C, ROWS, Wi)
        )
    return out


# revision 7
# speedup vs baseline: 1.0769x; 1.0769x over previous
"""Trainium2 Bass kernel for nn_DinoGazeSpade (segment_reduce + repaint).

reference semantics:
  seg_feat = mask[:, ::14, ::14]                       # nearest-downsample to 28x28
  seg_avg[b, s, :] = mean of feat pixels with seg==s   # scatter_mean over B*128 segments
  out[b, :, hi, wi] = seg_avg[b, mask[b, hi, wi], :]   # repaint at full res

Sharding: 8 cores = 2 batches x 4 row-slices of the 392-row full-res output.
Each core computes its batch's seg_avg table (tiny) and paints its 98-row
slice. The paint is a one-hot(segment) x seg_avg matmul on the tensor engine,
which directly produces the channel-major output layout.

The host ships the patch-pixel one-hots pre-scaled by 1/count (a bincount
over 784 indices per batch), so the scatter matmuls accumulate seg_avg
directly and the device never touches counts. Everything runs single-plane
bf16 and the output is written bf16 (cast to fp32 on the host): the 2e-2
rel-err budget dwarfs bf16 rounding (~6e-3 end to end).
"""

import numpy as np
import ml_dtypes
from contextlib import ExitStack

import concourse.bass as bass
import concourse.tile as tile
from concourse import bacc, mybir
from concourse.bass_utils import run_bass_kernel_spmd

# problem shape (hardcoded per contract)
B, C, Hp, Wp = 2, 768, 28, 28
Hi, Wi = 392, 392
S = 128                    # segments per image
N_CORES = 8
ROWS = Hi // 4             # 98 full-res rows per core
NPIX = ROWS * Wi           # 38416 pixels per core
NPATCH = Hp * Wp           # 784 patch pixels
PCHUNK = 112               # 784 = 7 * 112 patch-pixel chunks (partition dim)
NCH = NPATCH // PCHUNK     # 7 chunks
PTILE = 512                # paint pixel tile (one PSUM bank)
GROUP = 3 * PTILE          # 1536 pixels per paint group
NGROUP = NPIX // GROUP     # 25 full groups
REM = NPIX - NGROUP * GROUP  # 16 remainder pixels
CT = C // 128              # 6 channel tiles

f32 = mybir.dt.float32
bf16 = mybir.dt.bfloat16
i32 = mybir.dt.int32

_CACHED_NC = None


def _build_nc():
    nc = bacc.Bacc()
    fpk_hbm = nc.dram_tensor("fpk", [PCHUNK, NCH, C], bf16, kind="ExternalInput")
    ohk_hbm = nc.dram_tensor("ohk", [PCHUNK, NCH, S], bf16, kind="ExternalInput")
    mask_hbm = nc.dram_tensor("mask", [1, NPIX], bf16, kind="ExternalInput")
    out_hbm = nc.dram_tensor("out", [C, NPIX], bf16, kind="ExternalOutput")

    with tile.TileContext(nc) as tc, ExitStack() as ctx:
        const = ctx.enter_context(tc.tile_pool(name="const", bufs=1))
        segp = ctx.enter_context(tc.tile_pool(name="segp", bufs=1))
        # paint-phase pools created BEFORE the scatter scratch pool so
        # one-hot building can overlap the scatter phase
        sbB = ctx.enter_context(tc.tile_pool(name="sbB", bufs=9))
        osb = ctx.enter_context(tc.tile_pool(name="osb", bufs=10))
        psB = ctx.enter_context(tc.tile_pool(name="psB", bufs=2, space="PSUM"))

        # ---- input loads: scatter operands first (critical path), then the
        # full-res mask (needed slightly later by the paint) ----
        ld = ctx.enter_context(tc.tile_pool(name="ld", bufs=1))
        ohk = ld.tile([PCHUNK, NCH, S], bf16)
        nc.gpsimd.dma_start(out=ohk[:], in_=ohk_hbm[:, :, :])
        fsb = ld.tile([PCHUNK, NCH, C], bf16)
        # chunk 0 lands first so the first scatter matmul can start early
        nc.gpsimd.dma_start(out=fsb[:, 0:2, :], in_=fpk_hbm[:, 0:2, :])
        nc.gpsimd.dma_start(out=fsb[:, 2:NCH, :], in_=fpk_hbm[:, 2:NCH, :])
        mask_sb = ld.tile([1, NPIX], bf16)
        nc.gpsimd.dma_start(out=mask_sb[:], in_=mask_hbm[:, :])

        # ---- constants ----
        iota_pi = const.tile([128, 1], i32)           # partition index
        nc.gpsimd.iota(iota_pi[:], [[0, 1]], channel_multiplier=1)
        iota_pf = const.tile([128, 1], f32)
        nc.vector.tensor_copy(iota_pf[:], iota_pi[:])
        ones_bf = const.tile([1, 128], bf16)
        nc.vector.memset(ones_bf[:], 1.0)

        # ---- phase A: scatter-mean -> seg_avg [S=128, C] (one-hots are
        # pre-scaled by 1/count on the host) ----
        seg_bf = segp.tile([128, C], bf16)

        psA_cm = tc.tile_pool(name="psA", bufs=1, space="PSUM")
        with psA_cm as psA:
            sums0 = psA.tile([128, 384], f32, tag="sums0", name="sums0")
            sums1 = psA.tile([128, 384], f32, tag="sums1", name="sums1")
            for k in range(NCH):
                first, last = k == 0, k == NCH - 1
                nc.tensor.matmul(sums0[:], lhsT=ohk[:, k, :], rhs=fsb[:, k, 0:384],
                                 start=first, stop=last)
                nc.tensor.matmul(sums1[:], lhsT=ohk[:, k, :], rhs=fsb[:, k, 384:768],
                                 start=first, stop=last)
            nc.vector.tensor_copy(seg_bf[:, 0:384], sums0[:])
            nc.vector.tensor_copy(seg_bf[:, 384:768], sums1[:])

        # ---- phase B: paint full-res pixels ----
        # psO tiles span 3 PSUM banks: the 3 per-tile matmuls land in one
        # psO tile and drain with a single wide cast (amortizes the ~300ns
        # fixed cost per DVE/ACT instruction). psB 2x1 + psO 2x3 = 8 banks.
        psO = ctx.enter_context(tc.tile_pool(name="psO", bufs=2, space="PSUM"))

        def paint(pix0, sizes):
            # one group: pixels [pix0, pix0+sum(sizes)), one tile per size
            npx = sum(sizes)
            offs = [sum(sizes[:t]) for t in range(len(sizes))]
            ohs = []
            for t, sz in enumerate(sizes):
                bc = psB.tile([128, sz], f32, tag="bc", name="bc")
                nc.tensor.matmul(
                    bc[:], lhsT=ones_bf[:],
                    rhs=mask_sb[0:1, pix0 + offs[t]:pix0 + offs[t] + sz],
                    start=True, stop=True,
                )
                oh = sbB.tile([128, sz], bf16, tag="ohb", name="ohb")
                nc.vector.tensor_scalar(
                    out=oh[:], in0=bc[:], scalar1=iota_pf[:], scalar2=None,
                    op0=mybir.AluOpType.is_equal,
                )
                ohs.append(oh)
            for c in range(CT):
                ob = osb.tile([128, npx], bf16, tag="ob", name="ob")
                op = psO.tile([128, npx], f32, tag="op", name="op")
                for t in range(len(sizes)):
                    nc.tensor.matmul(op[:, offs[t]:offs[t] + sizes[t]],
                                     lhsT=seg_bf[:, c * 128:(c + 1) * 128],
                                     rhs=ohs[t][:], start=True, stop=True)
                # one wide psum->sbuf cast; DVE also builds one-hots, so it
                # gets 1/3 of the drain (evenly interleaved) and ACT the rest
                if c % 3 == 0:
                    nc.vector.tensor_copy(ob[:], op[:])
                else:
                    nc.scalar.copy(ob[:], op[:])
                nc.sync.dma_start(
                    out=out_hbm[c * 128:(c + 1) * 128, pix0:pix0 + npx], in_=ob[:]
                )

        for g in range(NGROUP):
            paint(g * GROUP, [PTILE] * 3)
        if REM:
            paint(NGROUP * GROUP, [REM])

    nc.compile()
    return nc


def make_in_maps(F_semantic_patches, segmentation_mask):
    F = np.asarray(F_semantic_patches, dtype=np.float32)
    M = np.asarray(segmentation_mask)
    in_maps = []
    seg_oh = {}
    for b in range(B):
        seg = np.clip(M[b, ::Hi // Hp, ::Wi // Wp].reshape(-1), 0, S - 1)  # [784]
        cnt = np.bincount(seg, minlength=S).astype(np.float32)
        rcp = 1.0 / np.maximum(cnt, 1.0)
        oh = (seg[:, None] == np.arange(S)[None, :]).astype(np.float32) * rcp[None, :]
        # [p, k, s] so one DMA lands chunk k on partitions
        seg_oh[b] = np.ascontiguousarray(
            oh.reshape(NCH, PCHUNK, S).transpose(1, 0, 2)
        ).astype(ml_dtypes.bfloat16)
    for core in range(N_CORES):
        b, q = divmod(core, 4)
        feat = F[b].reshape(C, NPATCH).T                               # [784, 768]
        fpk = np.ascontiguousarray(
            feat.reshape(NCH, PCHUNK, C).transpose(1, 0, 2)
        ).astype(ml_dtypes.bfloat16)
        mask = np.ascontiguousarray(
            M[b, q * ROWS:(q + 1) * ROWS, :].reshape(1, NPIX)
        ).astype(ml_dtypes.bfloat16)
        in_maps.append({"fpk": fpk, "ohk": seg_oh[b], "mask": mask})
    return in_maps


def kernel(F_semantic_patches: np.ndarray, segmentation_mask: np.ndarray) -> np.ndarray:
    global _CACHED_NC
    if _CACHED_NC is None:
        _CACHED_NC = _build_nc()
    nc = _CACHED_NC

    in_maps = make_in_maps(F_semantic_patches, segmentation_mask)

    res = run_bass_kernel_spmd(nc, in_maps, core_ids=list(range(N_CORES)))

    out = np.empty((B, C, Hi, Wi), dtype=np.float32)
    for core in range(N_CORES):
        b, q = divmod(core, 4)
        out[b, :, q * ROWS:(q + 1) * ROWS, :] = (
            res.results[core]["out"].astype(np.float32).reshape(C, ROWS, Wi)
        )
    return out


# revision 10
# speedup vs baseline: 1.0929x; 1.0149x over previous
"""Trainium2 Bass kernel for nn_DinoGazeSpade (segment_reduce + repaint).

reference semantics:
  seg_feat = mask[:, ::14, ::14]                       # nearest-downsample to 28x28
  seg_avg[b, s, :] = mean of feat pixels with seg==s   # scatter_mean over B*128 segments
  out[b, :, hi, wi] = seg_avg[b, mask[b, hi, wi], :]   # repaint at full res

Sharding: 8 cores = 2 batches x 4 row-slices of the 392-row full-res output.
Each core computes its batch's seg_avg table (tiny) and paints its 98-row
slice. The paint is a one-hot(segment) x seg_avg matmul on the tensor engine,
which directly produces the channel-major output layout.

The host ships the patch-pixel one-hots pre-scaled by 1/count (a bincount
over 784 indices per batch), so the scatter matmuls accumulate seg_avg
directly and the device never touches counts. Everything runs single-plane
bf16 and the output is written bf16 (cast to fp32 on the host): the 2e-2
rel-err budget dwarfs bf16 rounding (~6e-3 end to end).
"""

import numpy as np
import ml_dtypes
from contextlib import ExitStack

import concourse.bass as bass
import concourse.tile as tile
from concourse import bacc, mybir
from concourse.bass_utils import run_bass_kernel_spmd

# problem shape (hardcoded per contract)
B, C, Hp, Wp = 2, 768, 28, 28
Hi, Wi = 392, 392
S = 128                    # segments per image
N_CORES = 8
ROWS = Hi // 4             # 98 full-res rows per core
NPIX = ROWS * Wi           # 38416 pixels per core
NPATCH = Hp * Wp           # 784 patch pixels
PCHUNK = 112               # 784 = 7 * 112 patch-pixel chunks (partition dim)
NCH = NPATCH // PCHUNK     # 7 chunks
PTILE = 512                # paint pixel tile (one PSUM bank)
GROUP = 3 * PTILE          # 1536 pixels per paint group
NGROUP = NPIX // GROUP     # 25 full groups
REM = NPIX - NGROUP * GROUP  # 16 remainder pixels
CT = C // 128              # 6 channel tiles

f32 = mybir.dt.float32
bf16 = mybir.dt.bfloat16
i32 = mybir.dt.int32

_CACHED_NC = None


def _build_nc():
    nc = bacc.Bacc()
    fpk_hbm = nc.dram_tensor("fpk", [PCHUNK, NCH, C], bf16, kind="ExternalInput")
    ohk_hbm = nc.dram_tensor("ohk", [PCHUNK, NCH, S], bf16, kind="ExternalInput")
    mask_hbm = nc.dram_tensor("mask", [1, NPIX], bf16, kind="ExternalInput")
    out_hbm = nc.dram_tensor("out", [C, NPIX], bf16, kind="ExternalOutput")

    with tile.TileContext(nc) as tc, ExitStack() as ctx:
        const = ctx.enter_context(tc.tile_pool(name="const", bufs=1))
        segp = ctx.enter_context(tc.tile_pool(name="segp", bufs=1))
        # paint-phase pools created BEFORE the scatter scratch pool so
        # one-hot building can overlap the scatter phase
        sbB = ctx.enter_context(tc.tile_pool(name="sbB", bufs=12))
        osb = ctx.enter_context(tc.tile_pool(name="osb", bufs=10))
        psB = ctx.enter_context(tc.tile_pool(name="psB", bufs=2, space="PSUM"))

        # ---- input loads: scatter operands first (critical path), then the
        # full-res mask (needed slightly later by the paint) ----
        ld = ctx.enter_context(tc.tile_pool(name="ld", bufs=1))
        ohk = ld.tile([PCHUNK, NCH, S], bf16)
        nc.gpsimd.dma_start(out=ohk[:], in_=ohk_hbm[:, :, :])
        fsb = ld.tile([PCHUNK, NCH, C], bf16)
        # chunk 0 lands first so the first scatter matmul can start early;
        # the mask gates the prebuilt paint one-hots (first in PE order), so
        # it lands before the tail feature chunks
        nc.gpsimd.dma_start(out=fsb[:, 0:2, :], in_=fpk_hbm[:, 0:2, :])
        mask_sb = ld.tile([1, NPIX], bf16)
        nc.gpsimd.dma_start(out=mask_sb[:], in_=mask_hbm[:, :])
        nc.gpsimd.dma_start(out=fsb[:, 2:NCH, :], in_=fpk_hbm[:, 2:NCH, :])

        # ---- constants ----
        iota_pi = const.tile([128, 1], i32)           # partition index
        nc.gpsimd.iota(iota_pi[:], [[0, 1]], channel_multiplier=1)
        iota_pf = const.tile([128, 1], f32)
        nc.vector.tensor_copy(iota_pf[:], iota_pi[:])
        ones_bf = const.tile([1, 128], bf16)
        nc.vector.memset(ones_bf[:], 1.0)

        # ---- paint one-hot builder (used for prefetch + steady state) ----
        def build_ohs(pix0, sizes):
            ohs = []
            for t, sz in enumerate(sizes):
                off = sum(sizes[:t])
                bc = psB.tile([128, sz], f32, tag="bc", name="bc")
                nc.tensor.matmul(
                    bc[:], lhsT=ones_bf[:],
                    rhs=mask_sb[0:1, pix0 + off:pix0 + off + sz],
                    start=True, stop=True,
                )
                oh = sbB.tile([128, sz], bf16, tag="ohb", name="ohb")
                nc.vector.tensor_scalar(
                    out=oh[:], in0=bc[:], scalar1=iota_pf[:], scalar2=None,
                    op0=mybir.AluOpType.is_equal,
                )
                ohs.append(oh)
            return ohs

        # prebuild the first 3 groups' one-hots BEFORE the scatter matmuls in
        # PE program order: they only need the mask, so they run while the
        # feature chunks are still landing and phase A is reducing
        oh_q = [build_ohs(g * GROUP, [PTILE] * 3) for g in range(3)]

        # ---- phase A: scatter-mean -> seg_avg [S=128, C] (one-hots are
        # pre-scaled by 1/count on the host) ----
        seg_bf = segp.tile([128, C], bf16)

        psA_cm = tc.tile_pool(name="psA", bufs=1, space="PSUM")
        with psA_cm as psA:
            sums0 = psA.tile([128, 384], f32, tag="sums0", name="sums0")
            sums1 = psA.tile([128, 384], f32, tag="sums1", name="sums1")
            for k in range(NCH):
                first, last = k == 0, k == NCH - 1
                nc.tensor.matmul(sums0[:], lhsT=ohk[:, k, :], rhs=fsb[:, k, 0:384],
                                 start=first, stop=last)
                nc.tensor.matmul(sums1[:], lhsT=ohk[:, k, :], rhs=fsb[:, k, 384:768],
                                 start=first, stop=last)
            nc.vector.tensor_copy(seg_bf[:, 0:384], sums0[:])
            nc.scalar.copy(seg_bf[:, 384:768], sums1[:])

        # ---- phase B: paint full-res pixels ----
        # psO: 6 single-bank tiles -> rotation depth 6, so matmuls never wait
        # on a cast (the cast+semaphore latency exceeds what a 2-deep
        # rotation of wide tiles can hide). psB 2x1 + psO 6x1 = 8 banks.
        psO = ctx.enter_context(tc.tile_pool(name="psO", bufs=6, space="PSUM"))
        cast_i = [0]

        def paint(pix0, sizes, ohs):
            npx = sum(sizes)
            offs = [sum(sizes[:t]) for t in range(len(sizes))]
            for c in range(CT):
                ob = osb.tile([128, npx], bf16, tag="ob", name="ob")
                ops = [psO.tile([128, sz], f32, tag="op", name="op")
                       for sz in sizes]
                for t in range(len(sizes)):
                    nc.tensor.matmul(ops[t][:],
                                     lhsT=seg_bf[:, c * 128:(c + 1) * 128],
                                     rhs=ohs[t][:], start=True, stop=True)
                for t in range(len(sizes)):
                    # psum->sbuf drain: DVE also builds one-hots, so it gets
                    # 7/18 of the casts and ACT the rest, evenly interleaved
                    dst = ob[:, offs[t]:offs[t] + sizes[t]]
                    if (cast_i[0] * 7) % 18 < 7:
                        nc.vector.tensor_copy(dst, ops[t][:])
                    else:
                        nc.scalar.copy(dst, ops[t][:])
                    cast_i[0] += 1
                nc.sync.dma_start(
                    out=out_hbm[c * 128:(c + 1) * 128, pix0:pix0 + npx], in_=ob[:]
                )

        groups = [(g * GROUP, [PTILE] * 3) for g in range(NGROUP)]
        if REM:
            groups.append((NGROUP * GROUP, [REM]))
        for gi, (pix0, sizes) in enumerate(groups):
            # keep 3 groups of one-hots in flight
            if gi + 3 < len(groups):
                oh_q.append(build_ohs(*groups[gi + 3]))
            paint(pix0, sizes, oh_q[gi])

    nc.compile()
    return nc


def make_in_maps(F_semantic_patches, segmentation_mask):
    F = np.asarray(F_semantic_patches, dtype=np.float32)
    M = np.asarray(segmentation_mask)
    in_maps = []
    seg_oh = {}
    for b in range(B):
        seg = np.clip(M[b, ::Hi // Hp, ::Wi // Wp].reshape(-1), 0, S - 1)  # [784]
        cnt = np.bincount(seg, minlength=S).astype(np.float32)
        rcp = 1.0 / np.maximum(cnt, 1.0)
        oh = (seg[:, None] == np.arange(S)[None, :]).astype(np.float32) * rcp[None, :]
        # [p, k, s] so one DMA lands chunk k on partitions
        seg_oh[b] = np.ascontiguousarray(
            oh.reshape(NCH, PCHUNK, S).transpose(1, 0, 2)
        ).astype(ml_dtypes.bfloat16)
    for core in range(N_CORES):
        b, q = divmod(core, 4)
        feat = F[b].reshape(C, NPATCH).T                               # [784, 768]
        fpk = np.ascontiguousarray(
            feat.reshape(NCH, PCHUNK, C).transpose(1, 0, 2)
        ).astype(ml_dtypes.bfloat16)
        mask = np.ascontiguousarray(
            M[b, q * ROWS:(q + 1) * ROWS, :].reshape(1, NPIX)
        ).astype(ml_dtypes.bfloat16)
        in_maps.append({"fpk": fpk, "ohk": seg_oh[b], "mask": mask})
    return in_maps


def kernel(F_semantic_patches: np.ndarray, segmentation_mask: np.ndarray) -> np.ndarray:
    global _CACHED_NC
    if _CACHED_NC is None:
        _CACHED_NC = _build_nc()
    nc = _CACHED_NC

    in_maps = make_in_maps(F_semantic_patches, segmentation_mask)

    res = run_bass_kernel_spmd(nc, in_maps, core_ids=list(range(N_CORES)))

    out = np.empty((B, C, Hi, Wi), dtype=np.float32)
    for core in range(N_CORES):
        b, q = divmod(core, 4)
        out[b, :, q * ROWS:(q + 1) * ROWS, :] = (
            res.results[core]["out"].astype(np.float32).reshape(C, ROWS, Wi)
        )
    return out
